# revision 16
# baseline (speedup 1.0000x reference)
"""MultiHeadAttention (B=2, S=2048, D=1024, H=16, depth=64) on 8 trn2 cores.

Sharding: core c -> batch b=c//4, head-group g=c%4 (heads 4g..4g+3).
v2 design (single fused pipeline per core):
  - host pre-transposes inputs to feature-major bf16 xq/xk/xv [1024, 2048];
    weights bf16; biases fp32 column vectors [256, 1].
  - DMA order matched to first use: xk in 8 chunks (K proj streams with the
    transfers), xq and xv as single batched DMAs; weights batched per tensor
    on the SP queue.
  - Phase A: K proj kk-outer across 8 PSUM banks (stationary weight reused
    8x), then V proj kk-outer, then PE-transposes of V into seq-major
    interleaved tiles vI[sc] [128, 4, 65] with an all-ones column 64 per
    head (so attn@V also produces the softmax denominator), then Q(qj=0).
  - attention processes head PAIRS with a one-step software-pipeline skew
    (attn@V of step g-1 issues after scores/exp of step g).
    Q(qj+1) projection interleaves into (qj, hp=1) g-steps; output
    projection of qj interleaves into (qj+1, hp=0) g-steps.
  - exp is SPLIT between ACT (true Exp activation, scale 1/8) and DVE
    (Schraudolph bf16 trick: y_bits = u16(23.083*s + B) viewed as bf16),
    pattern-tunable; softmax denominator stays consistent since it sums the
    same quantized weights via the vI ones column.
  - normalization: reciprocal of ctx row 64 (DVE, f32r) -> rank-1 PE matmul
    broadcast -> multiply (DVE) into feature-major bf16 ctxN [256, 2048].
  - out projection -> psum -> ACT copy to SBUF -> DMA (gpsimd queue) ->
    outT partial [1024, 2048] fp32; host sums the 4 head-group partials per
    batch, transposes back, adds bo.
"""

import numpy as np
import ml_dtypes

B, S, D = 2, 2048, 1024
FG = 256  # features per core (4 heads x 64)

# Schraudolph bf16 exp: bits = u16(A8*scores + SCHR_B); scores scale 1/8 folded
SCHR_A = (1 << 7) / float(np.log(2.0)) * 0.125
SCHR_B = 127.0 * (1 << 7) - (1 << 7) * 0.043 + 0.5  # +0.5 if convert truncates

# which exp tiles go to DVE: (2*g + hh) % DVE_MOD < DVE_CNT
DVE_MOD, DVE_CNT = 2, 1

_compiled = None


def _build_program(repeat=1, do_proj=True, do_attn=True, do_exp=True,
                   do_xdma=True, schr_b=None):
    import concourse.bass as bass  # noqa: F401
    import concourse.tile as tile
    from concourse import bacc, mybir, masks

    f32 = mybir.dt.float32
    f32r = mybir.dt.float32r
    bf16 = mybir.dt.bfloat16
    u16 = mybir.dt.uint16
    EXP = mybir.ActivationFunctionType.Exp
    IDENT = mybir.ActivationFunctionType.Identity
    MULT = mybir.AluOpType.mult
    ADD = mybir.AluOpType.add
    schr_b_val = SCHR_B if schr_b is None else schr_b

    nc = bacc.Bacc("TRN2", target_bir_lowering=False, debug=False)

    xq_d = nc.dram_tensor("xq", [D, S], bf16, kind="ExternalInput")
    xk_d = nc.dram_tensor("xk", [D, S], bf16, kind="ExternalInput")
    xv_d = nc.dram_tensor("xv", [D, S], bf16, kind="ExternalInput")
    wq_d = nc.dram_tensor("wq", [D, FG], bf16, kind="ExternalInput")
    wk_d = nc.dram_tensor("wk", [D, FG], bf16, kind="ExternalInput")
    wv_d = nc.dram_tensor("wv", [D, FG], bf16, kind="ExternalInput")
    wo_d = nc.dram_tensor("wo", [FG, D], bf16, kind="ExternalInput")
    bq_d = nc.dram_tensor("bq", [FG, 1], f32, kind="ExternalInput")
    bk_d = nc.dram_tensor("bk", [FG, 1], f32, kind="ExternalInput")
    bv_d = nc.dram_tensor("bv", [FG, 1], f32, kind="ExternalInput")
    out_d = nc.dram_tensor("out", [D, S], f32, kind="ExternalOutput")

    def use_dve(qj, hp, g, hh):
        if not do_exp:
            return False
        return (2 * g + hh) % DVE_MOD < DVE_CNT

    with tile.TileContext(nc) as tc:
      for _rep in range(repeat):
        with tc.tile_pool(name="const", bufs=1) as cpool:
            onesf = cpool.tile([1, 512], f32, tag="onesf", name="onesf")
            nc.gpsimd.memset(onesf[:], 1.0)
            ones_r = cpool.tile([1, 64], f32r, tag="ones_r", name="ones_r")
            nc.vector.tensor_copy(ones_r[:], onesf[:, 0:64])
            o41f = cpool.tile([128, 4, 1], f32, tag="o41f", name="o41f")
            nc.gpsimd.memset(o41f[:], 1.0)
            ones41 = cpool.tile([128, 4, 1], bf16, tag="ones41", name="ones41")
            nc.vector.tensor_copy(ones41[:], o41f[:])
            zbias = cpool.tile([128, 1], f32, tag="zbias", name="zbias")
            nc.gpsimd.memset(zbias[:], 0.0)
            ident = cpool.tile([128, 128], bf16, tag="ident", name="ident")
            masks.make_identity(nc, ident[:])

            # ---- input DMAs, first-use order ----
            # x tensors on the Pool queue: xk split in 8 chunks (K proj
            # streams with the transfer), xq/xv as one batched DMA each.
            # weights/biases batched per tensor on the SP queue.
            x_c = {}
            for nm, d in (("xk", xk_d), ("xv", xv_d), ("xq", xq_d)):
                for kk in range(8):
                    t = cpool.tile([128, S], bf16, tag=f"{nm}c{kk}",
                                   name=f"{nm}c{kk}")
                    if do_xdma:
                        nc.gpsimd.dma_start(
                            t[:], d.ap()[128 * kk:128 * (kk + 1), :])
                    else:
                        nc.gpsimd.memset(t[:], 0.0)
                    x_c[(nm, kk)] = t

            def xap(nm, kk):
                return x_c[(nm, kk)][:]

            w_all = {}
            b_sb = {}
            w_all["wk"] = cpool.tile([128, 8, FG], bf16, tag="wka", name="wka")
            nc.sync.dma_start(w_all["wk"][:],
                              wk_d.ap().rearrange("(k p) c -> p k c", p=128))
            b_sb["bk"] = cpool.tile([128, 2, 1], f32, tag="bka", name="bka")
            nc.sync.dma_start(b_sb["bk"][:],
                              bk_d.ap().rearrange("(k p) c -> p k c", p=128))
            w_all["wv"] = cpool.tile([128, 8, FG], bf16, tag="wva", name="wva")
            nc.sync.dma_start(w_all["wv"][:],
                              wv_d.ap().rearrange("(k p) c -> p k c", p=128))
            b_sb["bv"] = cpool.tile([128, 2, 1], f32, tag="bva", name="bva")
            nc.sync.dma_start(b_sb["bv"][:],
                              bv_d.ap().rearrange("(k p) c -> p k c", p=128))
            w_all["wq"] = cpool.tile([128, 8, FG], bf16, tag="wqa", name="wqa")
            nc.sync.dma_start(w_all["wq"][:],
                              wq_d.ap().rearrange("(k p) c -> p k c", p=128))
            b_sb["bq"] = cpool.tile([128, 2, 1], f32, tag="bqa", name="bqa")
            nc.sync.dma_start(b_sb["bq"][:],
                              bq_d.ap().rearrange("(k p) c -> p k c", p=128))
            wo_all = cpool.tile([128, 2, D], bf16, tag="woa", name="woa")
            nc.sync.dma_start(wo_all[:],
                              wo_d.ap().rearrange("(k p) c -> p k c", p=128))

            def wap(nm, kk, pch):
                return w_all[nm][:, kk, 128 * pch:128 * (pch + 1)]

            def bap(nm, pch):
                return b_sb[nm][:, pch, :]

            # persistent activations
            qT = [cpool.tile([128, S], bf16, tag=f"qT{p}", name=f"qT{p}")
                  for p in range(2)]
            kT = [cpool.tile([128, S], bf16, tag=f"kT{p}", name=f"kT{p}")
                  for p in range(2)]
            vT = [cpool.tile([128, S], bf16, tag=f"vT{p}", name=f"vT{p}")
                  for p in range(2)]
            vI = [cpool.tile([128, 4, 65], bf16, tag=f"vI{sc}", name=f"vI{sc}")
                  for sc in range(16)]
            for sc in range(16):
                nc.vector.tensor_copy(vI[sc][:, :, 64:65], ones41[:])
            ctxN = [cpool.tile([128, S], bf16, tag=f"ctxN{p}", name=f"ctxN{p}")
                    for p in range(2)]

            # ---------------- phase A: K,V proj (kk-outer), transposes -----
            if do_proj:
                with tc.tile_pool(name="pa", bufs=1, space="PSUM") as papool:
                    for wnm, bnm, xnm, outT in (("wk", "bk", "xk", kT),
                                                ("wv", "bv", "xv", vT)):
                        ps = [papool.tile([128, 512], f32, name=f"pa{i}",
                                          bufs=1) for i in range(8)]
                        for kk in range(8):
                            for pch in range(2):
                                for qc in range(4):
                                    nc.tensor.matmul(
                                        ps[pch * 4 + qc][:],
                                        wap(wnm, kk, pch),
                                        xap(xnm, kk)[:, 512 * qc:512 * (qc + 1)],
                                        start=(kk == 0), stop=(kk == 7))
                        for pch in range(2):
                            for qc in range(4):
                                nc.scalar.activation(
                                    outT[pch][:, 512 * qc:512 * (qc + 1)],
                                    ps[pch * 4 + qc][:], IDENT,
                                    bias=bap(bnm, pch), scale=1.0)
                    # V transposes: 8 psum slots in flight
                    for sc in range(16):
                        for pch in range(2):
                            t = papool.tile([128, 512], f32,
                                            name=f"pa{(2 * sc + pch) % 8}",
                                            bufs=1)
                            tp = t[:, 0:64].bitcast(bf16)
                            nc.tensor.transpose(
                                tp, vT[pch][:, 128 * sc:128 * (sc + 1)],
                                ident[:])
                            nc.vector.tensor_copy(
                                vI[sc][:, 2 * pch:2 * pch + 2, 0:64],
                                tp.rearrange("p (h d) -> p h d", h=2))

            with tc.tile_pool(name="mp", bufs=1, space="PSUM") as mpool, \
                 tc.tile_pool(name="scp", bufs=1, space="PSUM") as scp, \
                 tc.tile_pool(name="cxp", bufs=1, space="PSUM") as cxp, \
                 tc.tile_pool(name="exa", bufs=8) as expa, \
                 tc.tile_pool(name="exd", bufs=8) as expd, \
                 tc.tile_pool(name="rcp", bufs=2) as rcpool, \
                 tc.tile_pool(name="csp", bufs=2) as cspool, \
                 tc.tile_pool(name="obp", bufs=2) as obpool:

                def proj_group(xnm, wnm, bnm, pch, qc, outT, bias_engine):
                    """[128,512] output chunk: 8 accumulating MMs + bias."""
                    p = mpool.tile([128, 512], f32, name="mp", bufs=2)
                    for kk in range(8):
                        nc.tensor.matmul(
                            p[:], wap(wnm, kk, pch),
                            xap(xnm, kk)[:, 512 * qc:512 * (qc + 1)],
                            start=(kk == 0), stop=(kk == 7))
                    dst = outT[pch][:, 512 * qc:512 * (qc + 1)]
                    if bias_engine == "act":
                        nc.scalar.activation(dst, p[:], IDENT,
                                             bias=bap(bnm, pch), scale=1.0)
                    else:
                        nc.vector.tensor_scalar_add(dst, p[:], bap(bnm, pch))

                def qproj_step(qj, g):
                    """two Q-proj MMs (+bias at group end) for chunk qj."""
                    pch = g // 4
                    if g % 4 == 0:
                        qproj_step.p = mpool.tile([128, 512], f32, name="mp",
                                                  bufs=2)
                    for kk in (2 * (g % 4), 2 * (g % 4) + 1):
                        nc.tensor.matmul(
                            qproj_step.p[:], wap("wq", kk, pch),
                            xap("xq", kk)[:, 512 * qj:512 * (qj + 1)],
                            start=(kk == 0), stop=(kk == 7))
                    if g % 4 == 3:
                        nc.vector.tensor_scalar_add(
                            qT[pch][:, 512 * qj:512 * (qj + 1)],
                            qproj_step.p[:], bap("bq", pch))

                def outproj_step(qj, m, tail=False):
                    """one 128-row chunk of the output projection of qj."""
                    if tail:
                        op = scp.tile([128, 512], f32, name="sup", bufs=2)
                    else:
                        op = mpool.tile([128, 512], f32, name="mp", bufs=2)
                    for kk2 in range(2):
                        nc.tensor.matmul(
                            op[:],
                            wo_all[:, kk2, 128 * m:128 * (m + 1)],
                            ctxN[kk2][:, 512 * qj:512 * (qj + 1)],
                            start=(kk2 == 0), stop=(kk2 == 1))
                    ob = obpool.tile([128, 512], f32, name="ob", bufs=4)
                    nc.scalar.copy(ob[:], op[:])
                    dma_eng = nc.gpsimd if m % 2 == 0 else nc.sync
                    dma_eng.dma_start(
                        out_d.ap()[128 * m:128 * (m + 1),
                                   512 * qj:512 * (qj + 1)],
                        ob[:])

                # Q(qj=0) projection
                if do_proj:
                    for pch in range(2):
                        proj_group("xq", "wq", "bq", pch, 0, qT, "act")

                # ---------------- attention + fused extra work ------------
                if do_attn:
                  for qj in range(4):
                    for hp in range(2):
                        pch = hp
                        ctxs = [cxp.tile([65, 512], f32, name=f"ctx{hh}",
                                         bufs=1) for hh in range(2)]
                        pend = None
                        for g in range(8):
                            if do_proj and hp == 1 and qj < 3:
                                qproj_step(qj + 1, g)
                            if hp == 0 and qj > 0 and g >= 1:
                                outproj_step(qj - 1, g - 1)

                            cur = []
                            for hh in range(2):
                                off = 64 * hh
                                sup = scp.tile([128, 2, 512], f32, name="sup",
                                               bufs=2)
                                for j in range(2):
                                    ki = 2 * g + j
                                    nc.tensor.matmul(
                                        sup[:, j, :],
                                        kT[pch][off:off + 64,
                                                128 * ki:128 * (ki + 1)],
                                        qT[pch][off:off + 64,
                                                512 * qj:512 * (qj + 1)],
                                        start=True, stop=True,
                                        tile_position=(off, 0))
                                if use_dve(qj, hp, g, hh):
                                    exd = expd.tile([128, 2, 512], u16,
                                                    name="exd", bufs=4)
                                    with nc.allow_low_precision(
                                            reason="schraudolph exp bits"):
                                        nc.vector.tensor_scalar(
                                            exd[:], sup[:],
                                            SCHR_A, schr_b_val,
                                            op0=MULT, op1=ADD)
                                    cur.append(exd[:].bitcast(bf16))
                                elif do_exp:
                                    exa = expa.tile([128, 2, 512], bf16,
                                                    name="exa", bufs=4)
                                    nc.scalar.activation(exa[:], sup[:], EXP,
                                                         bias=zbias[:],
                                                         scale=0.125)
                                    cur.append(exa[:])
                                else:
                                    cur.append(None)
                            if pend is not None:
                                for hh in range(2):
                                    for j in range(2):
                                        pk = 2 * (g - 1) + j
                                        mv = (pend[hh][:, j, :] if do_exp else
                                              qT[pch][:, 512 * qj:512 * (qj + 1)])
                                        nc.tensor.matmul(
                                            ctxs[hh][:],
                                            vI[pk][:, 2 * pch + hh, :],
                                            mv,
                                            start=(pk == 0), stop=False)
                            pend = cur
                        if hp == 0 and qj > 0:
                            outproj_step(qj - 1, 7)
                        # tail attn@V + normalization, fused per head so the
                        # reciprocal of head 0 runs while head 1's tail matmuls
                        # are still on the PE (no PE wait on DVE at block end);
                        # ctx values stage to SBUF on ACT in parallel.
                        for hh in range(2):
                            off = 64 * hh
                            for j in range(2):
                                pk = 14 + j
                                mv = (pend[hh][:, j, :] if do_exp else
                                      qT[pch][:, 512 * qj:512 * (qj + 1)])
                                nc.tensor.matmul(
                                    ctxs[hh][:], vI[pk][:, 2 * pch + hh, :], mv,
                                    start=False, stop=(pk == 15))
                            rc = rcpool.tile([1, 512], f32r, name="rc", bufs=2)
                            with nc.allow_low_precision(
                                    reason="f32r for PE broadcast"):
                                nc.vector.reciprocal(rc[:], ctxs[hh][64:65, :])
                            cs = cspool.tile([64, 512], f32, name="cs", bufs=4)
                            nc.scalar.copy(cs[:], ctxs[hh][0:64, :])
                            bc = scp.tile([64, 512], f32, name="sup", bufs=2)
                            nc.tensor.matmul(bc[:], ones_r[:, :], rc[:],
                                             start=True, stop=True)
                            nc.vector.tensor_tensor(
                                ctxN[pch][off:off + 64,
                                          512 * qj:512 * (qj + 1)],
                                cs[:], bc[:], MULT)
                  # tail: output projection of the last qj chunk
                  for m in range(8):
                      outproj_step(3, m, tail=True)

    nc.compile()
    return nc


def _make_in_maps(q, k, v, wq, bq, wk, bk, wv, bv, wo):
    bf = ml_dtypes.bfloat16
    in_maps = []
    for c in range(8):
        b, g = divmod(c, 4)
        fs = slice(FG * g, FG * (g + 1))
        in_maps.append({
            "xq": np.ascontiguousarray(q[b].T.astype(bf)),
            "xk": np.ascontiguousarray(k[b].T.astype(bf)),
            "xv": np.ascontiguousarray(v[b].T.astype(bf)),
            "wq": np.ascontiguousarray(wq[fs, :].T.astype(bf)),
            "wk": np.ascontiguousarray(wk[fs, :].T.astype(bf)),
            "wv": np.ascontiguousarray(wv[fs, :].T.astype(bf)),
            "wo": np.ascontiguousarray(wo[:, fs].T.astype(bf)),
            "bq": np.ascontiguousarray(bq[fs].reshape(FG, 1).astype(np.float32)),
            "bk": np.ascontiguousarray(bk[fs].reshape(FG, 1).astype(np.float32)),
            "bv": np.ascontiguousarray(bv[fs].reshape(FG, 1).astype(np.float32)),
        })
    return in_maps


def kernel(q, k, v, wq, bq, wk, bk, wv, bv, wo, bo):
    from concourse.bass_utils import run_bass_kernel_spmd

    global _compiled
    if _compiled is None:
        _compiled = _build_program()
    nc = _compiled

    args = [np.asarray(a, dtype=np.float32)
            for a in (q, k, v, wq, bq, wk, bk, wv, bv, wo)]
    bo = np.asarray(bo, dtype=np.float32)
    in_maps = _make_in_maps(*args)
    res = run_bass_kernel_spmd(nc, in_maps, core_ids=list(range(8)))
    outs = [np.asarray(res.results[c]["out"]) for c in range(8)]
    full = []
    for b in range(B):
        acc = outs[4 * b] + outs[4 * b + 1] + outs[4 * b + 2] + outs[4 * b + 3]
        full.append(acc.T + bo[None, :])
    return np.stack(full).astype(np.float32)


# revision 22
# speedup vs baseline: 1.3789x; 1.3789x over previous
"""MultiHeadAttention (B=2, S=2048, D=1024, H=16, depth=64) on 8 trn2 cores.

Sharding: core c -> batch b=c//4, head-group g=c%4 (heads 4g..4g+3).
v2 design (single fused pipeline per core):
  - host pre-transposes inputs to feature-major bf16 xq/xk/xv [1024, 2048];
    weights bf16; biases fp32 column vectors [256, 1].
  - DMA order matched to first use: xk in 8 chunks (K proj streams with the
    transfers), xq and xv as single batched DMAs; weights batched per tensor
    on the SP queue.
  - Phase A: K proj kk-outer across 8 PSUM banks (stationary weight reused
    8x), then V proj kk-outer, then PE-transposes of V into seq-major
    interleaved tiles vI[sc] [128, 4, 65] with an all-ones column 64 per
    head (so attn@V also produces the softmax denominator), then Q(qj=0).
  - attention processes head PAIRS with a one-step software-pipeline skew
    (attn@V of step g-1 issues after scores/exp of step g).
    Q(qj+1) projection interleaves into (qj, hp=1) g-steps; output
    projection of qj interleaves into (qj+1, hp=0) g-steps.
  - exp is SPLIT between ACT (true Exp activation, scale 1/8) and DVE
    (Schraudolph bf16 trick: y_bits = u16(23.083*s + B) viewed as bf16),
    pattern-tunable; softmax denominator stays consistent since it sums the
    same quantized weights via the vI ones column.
  - normalization: reciprocal of ctx row 64 (DVE, f32r) -> rank-1 PE matmul
    broadcast -> multiply (DVE) into feature-major bf16 ctxN [256, 2048].
  - out projection -> psum -> ACT copy to SBUF -> DMA (gpsimd queue) ->
    outT partial [1024, 2048] fp32; host sums the 4 head-group partials per
    batch, transposes back, adds bo.
"""

import numpy as np
import ml_dtypes

B, S, D = 2, 2048, 1024
FG = 256  # features per core (4 heads x 64)

# Schraudolph bf16 exp: bits = u16(A8*scores + SCHR_B); scores scale 1/8 folded
SCHR_A = (1 << 7) / float(np.log(2.0)) * 0.125
SCHR_B = 127.0 * (1 << 7) - (1 << 7) * 0.043 + 0.5  # +0.5 if convert truncates

# which exp tiles go to DVE: (2*g + hh) % DVE_MOD < DVE_CNT
DVE_MOD, DVE_CNT = 2, 1

_compiled = None


def _build_program(repeat=1, do_proj=True, do_attn=True, do_exp=True,
                   do_xdma=True, schr_b=None, rt_attnv=False, rt_proj=False):
    import concourse.bass as bass  # noqa: F401
    import concourse.tile as tile
    from concourse import bacc, mybir, masks

    f32 = mybir.dt.float32
    f32r = mybir.dt.float32r
    bf16 = mybir.dt.bfloat16
    u16 = mybir.dt.uint16
    EXP = mybir.ActivationFunctionType.Exp
    IDENT = mybir.ActivationFunctionType.Identity
    MULT = mybir.AluOpType.mult
    ADD = mybir.AluOpType.add
    schr_b_val = SCHR_B if schr_b is None else schr_b

    nc = bacc.Bacc("TRN2", target_bir_lowering=False, debug=False)

    xq_d = nc.dram_tensor("xq", [D, S], bf16, kind="ExternalInput")
    xk_d = nc.dram_tensor("xk", [D, S], bf16, kind="ExternalInput")
    xv_d = nc.dram_tensor("xv", [D, S], bf16, kind="ExternalInput")
    wq_d = nc.dram_tensor("wq", [D, FG], bf16, kind="ExternalInput")
    wk_d = nc.dram_tensor("wk", [D, FG], bf16, kind="ExternalInput")
    wv_d = nc.dram_tensor("wv", [D, FG], bf16, kind="ExternalInput")
    wo_d = nc.dram_tensor("wo", [FG, D], bf16, kind="ExternalInput")
    bq_d = nc.dram_tensor("bq", [FG, 1], f32, kind="ExternalInput")
    bk_d = nc.dram_tensor("bk", [FG, 1], f32, kind="ExternalInput")
    bv_d = nc.dram_tensor("bv", [FG, 1], f32, kind="ExternalInput")
    out_d = nc.dram_tensor("out", [D, S], f32, kind="ExternalOutput")

    def use_dve(qj, hp, g, hh):
        if not do_exp:
            return False
        return (2 * g + hh) % DVE_MOD < DVE_CNT

    with tile.TileContext(nc) as tc:
      for _rep in range(repeat):
        with tc.tile_pool(name="const", bufs=1) as cpool:
            onesf = cpool.tile([1, 512], f32, tag="onesf", name="onesf")
            nc.gpsimd.memset(onesf[:], 1.0)
            ones_r = cpool.tile([1, 64], f32r, tag="ones_r", name="ones_r")
            nc.vector.tensor_copy(ones_r[:], onesf[:, 0:64])
            o41f = cpool.tile([128, 4, 1], f32, tag="o41f", name="o41f")
            nc.gpsimd.memset(o41f[:], 1.0)
            ones41 = cpool.tile([128, 4, 1], bf16, tag="ones41", name="ones41")
            nc.vector.tensor_copy(ones41[:], o41f[:])
            zbias = cpool.tile([128, 1], f32, tag="zbias", name="zbias")
            nc.gpsimd.memset(zbias[:], 0.0)
            ident = cpool.tile([128, 128], bf16, tag="ident", name="ident")
            masks.make_identity(nc, ident[:])

            # ---- input DMAs, first-use order ----
            # x tensors on the Pool queue: xk split in 8 chunks (K proj
            # streams with the transfer), xq/xv as one batched DMA each.
            # weights/biases batched per tensor on the SP queue.
            x_c = {}
            for nm, d in (("xk", xk_d), ("xv", xv_d), ("xq", xq_d)):
                for kk in range(8):
                    t = cpool.tile([128, S], bf16, tag=f"{nm}c{kk}",
                                   name=f"{nm}c{kk}")
                    if do_xdma:
                        nc.gpsimd.dma_start(
                            t[:], d.ap()[128 * kk:128 * (kk + 1), :])
                    else:
                        nc.gpsimd.memset(t[:], 0.0)
                    x_c[(nm, kk)] = t

            def xap(nm, kk):
                return x_c[(nm, kk)][:]

            w_all = {}
            b_sb = {}
            w_all["wk"] = cpool.tile([128, 8, FG], bf16, tag="wka", name="wka")
            nc.sync.dma_start(w_all["wk"][:],
                              wk_d.ap().rearrange("(k p) c -> p k c", p=128))
            b_sb["bk"] = cpool.tile([128, 2, 1], f32, tag="bka", name="bka")
            nc.sync.dma_start(b_sb["bk"][:],
                              bk_d.ap().rearrange("(k p) c -> p k c", p=128))
            w_all["wv"] = cpool.tile([128, 8, FG], bf16, tag="wva", name="wva")
            nc.sync.dma_start(w_all["wv"][:],
                              wv_d.ap().rearrange("(k p) c -> p k c", p=128))
            b_sb["bv"] = cpool.tile([128, 2, 1], f32, tag="bva", name="bva")
            nc.sync.dma_start(b_sb["bv"][:],
                              bv_d.ap().rearrange("(k p) c -> p k c", p=128))
            w_all["wq"] = cpool.tile([128, 8, FG], bf16, tag="wqa", name="wqa")
            nc.sync.dma_start(w_all["wq"][:],
                              wq_d.ap().rearrange("(k p) c -> p k c", p=128))
            b_sb["bq"] = cpool.tile([128, 2, 1], f32, tag="bqa", name="bqa")
            nc.sync.dma_start(b_sb["bq"][:],
                              bq_d.ap().rearrange("(k p) c -> p k c", p=128))
            wo_all = cpool.tile([128, 2, D], bf16, tag="woa", name="woa")
            nc.sync.dma_start(wo_all[:],
                              wo_d.ap().rearrange("(k p) c -> p k c", p=128))

            def wap(nm, kk, pch):
                return w_all[nm][:, kk, 128 * pch:128 * (pch + 1)]

            def bap(nm, pch):
                return b_sb[nm][:, pch, :]

            # persistent activations
            qT = [cpool.tile([128, S], bf16, tag=f"qT{p}", name=f"qT{p}")
                  for p in range(2)]
            kT = [cpool.tile([128, S], bf16, tag=f"kT{p}", name=f"kT{p}")
                  for p in range(2)]
            vT = [cpool.tile([128, S], bf16, tag=f"vT{p}", name=f"vT{p}")
                  for p in range(2)]
            vI = [cpool.tile([128, 4, 65], bf16, tag=f"vI{sc}", name=f"vI{sc}")
                  for sc in range(16)]
            for sc in range(16):
                nc.vector.tensor_copy(vI[sc][:, :, 64:65], ones41[:])
            ctxN = [cpool.tile([128, S], bf16, tag=f"ctxN{p}", name=f"ctxN{p}")
                    for p in range(2)]

            # ---------------- phase A: K,V proj (kk-outer), transposes -----
            if do_proj:
                with tc.tile_pool(name="pa", bufs=1, space="PSUM") as papool:
                    for wnm, bnm, xnm, outT in (("wk", "bk", "xk", kT),
                                                ("wv", "bv", "xv", vT)):
                        ps = [papool.tile([128, 512], f32, name=f"pa{i}",
                                          bufs=1) for i in range(8)]
                        for kk in range(8):
                            for pch in range(2):
                                for qc in range(4):
                                    nc.tensor.matmul(
                                        ps[pch * 4 + qc][:],
                                        wap(wnm, kk, pch),
                                        xap(xnm, kk)[:, 512 * qc:512 * (qc + 1)],
                                        start=(kk == 0), stop=(kk == 7))
                        for pch in range(2):
                            for qc in range(4):
                                nc.scalar.activation(
                                    outT[pch][:, 512 * qc:512 * (qc + 1)],
                                    ps[pch * 4 + qc][:], IDENT,
                                    bias=bap(bnm, pch), scale=1.0)
                    # V transposes: 8 psum slots in flight
                    for sc in range(16):
                        for pch in range(2):
                            t = papool.tile([128, 512], f32,
                                            name=f"pa{(2 * sc + pch) % 8}",
                                            bufs=1)
                            tp = t[:, 0:64].bitcast(bf16)
                            nc.tensor.transpose(
                                tp, vT[pch][:, 128 * sc:128 * (sc + 1)],
                                ident[:])
                            nc.vector.tensor_copy(
                                vI[sc][:, 2 * pch:2 * pch + 2, 0:64],
                                tp.rearrange("p (h d) -> p h d", h=2))

            with tc.tile_pool(name="mp", bufs=1, space="PSUM") as mpool, \
                 tc.tile_pool(name="scp", bufs=1, space="PSUM") as scp, \
                 tc.tile_pool(name="cxp", bufs=1, space="PSUM") as cxp, \
                 tc.tile_pool(name="exa", bufs=8) as expa, \
                 tc.tile_pool(name="exd", bufs=8) as expd, \
                 tc.tile_pool(name="rcp", bufs=2) as rcpool, \
                 tc.tile_pool(name="csp", bufs=2) as cspool, \
                 tc.tile_pool(name="obp", bufs=2) as obpool:

                def proj_group(xnm, wnm, bnm, pch, qc, outT, bias_engine):
                    """[128,512] output chunk: 8 accumulating MMs + bias."""
                    p = mpool.tile([128, 512], f32, name="mp", bufs=2)
                    for kk in range(8):
                        if rt_proj:
                            for h in range(2):
                                nc.tensor.matmul(
                                    p[:],
                                    wap(wnm, kk, pch)[64 * h:64 * h + 64, :],
                                    xap(xnm, kk)[64 * h:64 * h + 64,
                                                 512 * qc:512 * (qc + 1)],
                                    start=(kk == 0 and h == 0),
                                    stop=(kk == 7 and h == 1),
                                    tile_position=(64 * h, 0))
                            continue
                        nc.tensor.matmul(
                            p[:], wap(wnm, kk, pch),
                            xap(xnm, kk)[:, 512 * qc:512 * (qc + 1)],
                            start=(kk == 0), stop=(kk == 7))
                    dst = outT[pch][:, 512 * qc:512 * (qc + 1)]
                    if bias_engine == "act":
                        nc.scalar.activation(dst, p[:], IDENT,
                                             bias=bap(bnm, pch), scale=1.0)
                    else:
                        nc.vector.tensor_scalar_add(dst, p[:], bap(bnm, pch))

                def qproj_step(qj, g):
                    """two Q-proj MMs (+bias at group end) for chunk qj."""
                    pch = g // 4
                    if g % 4 == 0:
                        qproj_step.p = mpool.tile([128, 512], f32, name="mp",
                                                  bufs=2)
                    for kk in (2 * (g % 4), 2 * (g % 4) + 1):
                        nc.tensor.matmul(
                            qproj_step.p[:], wap("wq", kk, pch),
                            xap("xq", kk)[:, 512 * qj:512 * (qj + 1)],
                            start=(kk == 0), stop=(kk == 7))
                    if g % 4 == 3:
                        nc.vector.tensor_scalar_add(
                            qT[pch][:, 512 * qj:512 * (qj + 1)],
                            qproj_step.p[:], bap("bq", pch))

                def outproj_step(qj, m, tail=False):
                    """one 128-row chunk of the output projection of qj."""
                    if tail:
                        op = scp.tile([128, 512], f32, name="sup", bufs=2)
                    else:
                        op = mpool.tile([128, 512], f32, name="mp", bufs=2)
                    for kk2 in range(2):
                        nc.tensor.matmul(
                            op[:],
                            wo_all[:, kk2, 128 * m:128 * (m + 1)],
                            ctxN[kk2][:, 512 * qj:512 * (qj + 1)],
                            start=(kk2 == 0), stop=(kk2 == 1))
                    ob = obpool.tile([128, 512], f32, name="ob", bufs=4)
                    nc.scalar.copy(ob[:], op[:])
                    dma_eng = nc.gpsimd if m % 2 == 0 else nc.sync
                    dma_eng.dma_start(
                        out_d.ap()[128 * m:128 * (m + 1),
                                   512 * qj:512 * (qj + 1)],
                        ob[:])

                def attnv_mm(ctx, pk, pch, hh, mv, start, stop):
                    if rt_attnv:
                        for h in range(2):
                            nc.tensor.matmul(
                                ctx[:],
                                vI[pk][64 * h:64 * h + 64, 2 * pch + hh, :],
                                mv[64 * h:64 * h + 64, :],
                                start=(start and h == 0),
                                stop=(stop and h == 1),
                                tile_position=(64 * h, 0))
                    else:
                        nc.tensor.matmul(ctx[:], vI[pk][:, 2 * pch + hh, :],
                                         mv, start=start, stop=stop)

                # Q(qj=0) projection
                if do_proj:
                    for pch in range(2):
                        proj_group("xq", "wq", "bq", pch, 0, qT, "act")

                # ---------------- attention + fused extra work ------------
                if do_attn:
                  for qj in range(4):
                    for hp in range(2):
                        pch = hp
                        ctxs = [cxp.tile([65, 512], f32, name=f"ctx{hh}",
                                         bufs=1) for hh in range(2)]
                        pend = None
                        for g in range(8):
                            if do_proj and hp == 1 and qj < 3:
                                qproj_step(qj + 1, g)
                            if hp == 0 and qj > 0 and g >= 1:
                                outproj_step(qj - 1, g - 1)

                            cur = []
                            sups = [scp.tile([128, 2, 512], f32, name="sup",
                                             bufs=2) for _ in range(2)]
                            # j-outer, hh-inner: adjacent matmuls target
                            # disjoint PE row groups, so each LDWEIGHTS
                            # overlaps the other head's in-flight matmul
                            for j in range(2):
                                ki = 2 * g + j
                                for hh in range(2):
                                    off = 64 * hh
                                    nc.tensor.matmul(
                                        sups[hh][:, j, :],
                                        kT[pch][off:off + 64,
                                                128 * ki:128 * (ki + 1)],
                                        qT[pch][off:off + 64,
                                                512 * qj:512 * (qj + 1)],
                                        start=True, stop=True,
                                        tile_position=(off, 0))
                            for hh in range(2):
                                sup = sups[hh]
                                if use_dve(qj, hp, g, hh):
                                    exd = expd.tile([128, 2, 512], u16,
                                                    name="exd", bufs=4)
                                    with nc.allow_low_precision(
                                            reason="schraudolph exp bits"):
                                        nc.vector.tensor_scalar(
                                            exd[:], sup[:],
                                            SCHR_A, schr_b_val,
                                            op0=MULT, op1=ADD)
                                    cur.append(exd[:].bitcast(bf16))
                                elif do_exp:
                                    exa = expa.tile([128, 2, 512], bf16,
                                                    name="exa", bufs=4)
                                    nc.scalar.activation(exa[:], sup[:], EXP,
                                                         bias=zbias[:],
                                                         scale=0.125)
                                    cur.append(exa[:])
                                else:
                                    cur.append(None)
                            if pend is not None:
                                for hh in range(2):
                                    for j in range(2):
                                        pk = 2 * (g - 1) + j
                                        mv = (pend[hh][:, j, :] if do_exp else
                                              qT[pch][:, 512 * qj:512 * (qj + 1)])
                                        attnv_mm(ctxs[hh], pk, pch, hh, mv,
                                                 start=(pk == 0), stop=False)
                            pend = cur
                        if hp == 0 and qj > 0:
                            outproj_step(qj - 1, 7)
                        # tail attn@V + normalization, fused per head so the
                        # reciprocal of head 0 runs while head 1's tail matmuls
                        # are still on the PE (no PE wait on DVE at block end);
                        # ctx values stage to SBUF on ACT in parallel.
                        for hh in range(2):
                            off = 64 * hh
                            for j in range(2):
                                pk = 14 + j
                                mv = (pend[hh][:, j, :] if do_exp else
                                      qT[pch][:, 512 * qj:512 * (qj + 1)])
                                attnv_mm(ctxs[hh], pk, pch, hh, mv,
                                         start=False, stop=(pk == 15))
                            rc = rcpool.tile([1, 512], f32r, name="rc", bufs=2)
                            with nc.allow_low_precision(
                                    reason="f32r for PE broadcast"):
                                nc.vector.reciprocal(rc[:], ctxs[hh][64:65, :])
                            cs = cspool.tile([64, 512], f32, name="cs", bufs=4)
                            nc.scalar.copy(cs[:], ctxs[hh][0:64, :])
                            bc = scp.tile([64, 512], f32, name="sup", bufs=2)
                            nc.tensor.matmul(bc[:], ones_r[:, :], rc[:],
                                             start=True, stop=True)
                            nc.vector.tensor_tensor(
                                ctxN[pch][off:off + 64,
                                          512 * qj:512 * (qj + 1)],
                                cs[:], bc[:], MULT)
                  # tail: output projection of the last qj chunk
                  for m in range(8):
                      outproj_step(3, m, tail=True)

    nc.compile()
    return nc


def _make_in_maps(q, k, v, wq, bq, wk, bk, wv, bv, wo):
    bf = ml_dtypes.bfloat16
    in_maps = []
    for c in range(8):
        b, g = divmod(c, 4)
        fs = slice(FG * g, FG * (g + 1))
        in_maps.append({
            "xq": np.ascontiguousarray(q[b].T.astype(bf)),
            "xk": np.ascontiguousarray(k[b].T.astype(bf)),
            "xv": np.ascontiguousarray(v[b].T.astype(bf)),
            "wq": np.ascontiguousarray(wq[fs, :].T.astype(bf)),
            "wk": np.ascontiguousarray(wk[fs, :].T.astype(bf)),
            "wv": np.ascontiguousarray(wv[fs, :].T.astype(bf)),
            "wo": np.ascontiguousarray(wo[:, fs].T.astype(bf)),
            "bq": np.ascontiguousarray(bq[fs].reshape(FG, 1).astype(np.float32)),
            "bk": np.ascontiguousarray(bk[fs].reshape(FG, 1).astype(np.float32)),
            "bv": np.ascontiguousarray(bv[fs].reshape(FG, 1).astype(np.float32)),
        })
    return in_maps


def kernel(q, k, v, wq, bq, wk, bk, wv, bv, wo, bo):
    from concourse.bass_utils import run_bass_kernel_spmd

    global _compiled
    if _compiled is None:
        _compiled = _build_program()
    nc = _compiled

    args = [np.asarray(a, dtype=np.float32)
            for a in (q, k, v, wq, bq, wk, bk, wv, bv, wo)]
    bo = np.asarray(bo, dtype=np.float32)
    in_maps = _make_in_maps(*args)
    res = run_bass_kernel_spmd(nc, in_maps, core_ids=list(range(8)))
    outs = [np.asarray(res.results[c]["out"]) for c in range(8)]
    full = []
    for b in range(B):
        acc = outs[4 * b] + outs[4 * b + 1] + outs[4 * b + 2] + outs[4 * b + 3]
        full.append(acc.T + bo[None, :])
    return np.stack(full).astype(np.float32)


# revision 31
# speedup vs baseline: 4.1720x; 3.0256x over previous
"""MultiHeadAttention (B=2, S=2048, D=1024, H=16, depth=64) on 8 trn2 cores.

Sharding: core c -> batch b=c//4, head-group g=c%4 (heads 4g..4g+3).
v2 design (single fused pipeline per core):
  - host pre-transposes inputs to feature-major bf16 xq/xk/xv [1024, 2048];
    weights bf16; biases fp32 column vectors [256, 1].
  - DMA order matched to first use: xk in 8 chunks (K proj streams with the
    transfers), xq and xv as single batched DMAs; weights batched per tensor
    on the SP queue.
  - Phase A: K proj kk-outer across 8 PSUM banks (stationary weight reused
    8x), then V proj kk-outer, then PE-transposes of V into seq-major
    interleaved tiles vI[sc] [128, 4, 65] with an all-ones column 64 per
    head (so attn@V also produces the softmax denominator), then Q(qj=0).
  - attention processes head PAIRS with a one-step software-pipeline skew
    (attn@V of step g-1 issues after scores/exp of step g).
    Q(qj+1) projection interleaves into (qj, hp=1) g-steps; output
    projection of qj interleaves into (qj+1, hp=0) g-steps.
  - exp is SPLIT between ACT (true Exp activation, scale 1/8) and DVE
    (Schraudolph bf16 trick: y_bits = u16(23.083*s + B) viewed as bf16),
    pattern-tunable; softmax denominator stays consistent since it sums the
    same quantized weights via the vI ones column.
  - normalization: reciprocal of ctx row 64 (DVE, f32r) -> rank-1 PE matmul
    broadcast -> multiply (DVE) into feature-major bf16 ctxN [256, 2048].
  - out projection -> psum -> ACT copy to SBUF -> DMA (gpsimd queue) ->
    outT partial [1024, 2048] fp32; host sums the 4 head-group partials per
    batch, transposes back, adds bo.
"""

import numpy as np
import ml_dtypes

B, S, D = 2, 2048, 1024
FG = 256  # features per core (4 heads x 64)

# Schraudolph bf16 exp: bits = u16(A8*scores + SCHR_B); scores scale 1/8 folded
SCHR_A = (1 << 7) / float(np.log(2.0)) * 0.125
SCHR_B = 127.0 * (1 << 7) - (1 << 7) * 0.043 + 0.5  # +0.5 if convert truncates

# which exp tiles go to DVE: (2*g + hh) % DVE_MOD < DVE_CNT
DVE_MOD, DVE_CNT = 2, 1

_compiled = None


def _build_program(repeat=1, do_proj=True, do_attn=True, do_exp=True,
                   do_xdma=True, schr_b=None, rt_attnv=False, rt_proj=False):
    import concourse.bass as bass  # noqa: F401
    import concourse.tile as tile
    from concourse import bacc, mybir, masks

    f32 = mybir.dt.float32
    f32r = mybir.dt.float32r
    bf16 = mybir.dt.bfloat16
    u16 = mybir.dt.uint16
    EXP = mybir.ActivationFunctionType.Exp
    IDENT = mybir.ActivationFunctionType.Identity
    MULT = mybir.AluOpType.mult
    ADD = mybir.AluOpType.add
    schr_b_val = SCHR_B if schr_b is None else schr_b

    nc = bacc.Bacc("TRN2", target_bir_lowering=False, debug=False)

    xq_d = nc.dram_tensor("xq", [D, S], bf16, kind="ExternalInput")
    xk_d = nc.dram_tensor("xk", [D, S], bf16, kind="ExternalInput")
    xv_d = nc.dram_tensor("xv", [D, S], bf16, kind="ExternalInput")
    wq_d = nc.dram_tensor("wq", [D, FG], bf16, kind="ExternalInput")
    wk_d = nc.dram_tensor("wk", [D, FG], bf16, kind="ExternalInput")
    wv_d = nc.dram_tensor("wv", [D, FG], bf16, kind="ExternalInput")
    wo_d = nc.dram_tensor("wo", [FG, D], bf16, kind="ExternalInput")
    bq_d = nc.dram_tensor("bq", [FG, 1], f32, kind="ExternalInput")
    bk_d = nc.dram_tensor("bk", [FG, 1], f32, kind="ExternalInput")
    bv_d = nc.dram_tensor("bv", [FG, 1], f32, kind="ExternalInput")
    out_d = nc.dram_tensor("out", [D, S], f32, kind="ExternalOutput")

    def use_dve(qj, hp, g, hh):
        if not do_exp:
            return False
        return (2 * g + hh) % DVE_MOD < DVE_CNT

    with tile.TileContext(nc) as tc:
      for _rep in range(repeat):
        with tc.tile_pool(name="const", bufs=1) as cpool:
            onesf = cpool.tile([1, 512], f32, tag="onesf", name="onesf")
            nc.gpsimd.memset(onesf[:], 1.0)
            # block stationary for the merged per-block-pair denominator
            # broadcast: row 0 -> cols 0..63, row 64 -> cols 64..127
            # (memset can't target f32r: build in f32, tensor_copy across)
            ones2f = cpool.tile([65, 512], f32, tag="ones2f", name="ones2f")
            nc.gpsimd.memset(ones2f[:], 0.0)
            nc.gpsimd.memset(ones2f[0:1, 0:64], 1.0)
            nc.gpsimd.memset(ones2f[64:65, 64:128], 1.0)
            ones2 = cpool.tile([65, 128], f32r, tag="ones2", name="ones2")
            nc.vector.tensor_copy(ones2[:], ones2f[:, 0:128])
            # pre-zeroed reciprocal carriers (rows 1..63 stay zero forever)
            rc01 = []
            for i in range(2):
                t = cpool.tile([65, 512], f32r, tag=f"rc01{i}", name=f"rc01{i}")
                nc.vector.tensor_copy(t[:], ones2f[:, 0:512])
                rc01.append(t)
            o41f = cpool.tile([128, 4, 1], f32, tag="o41f", name="o41f")
            nc.gpsimd.memset(o41f[:], 1.0)
            ones41 = cpool.tile([128, 4, 1], bf16, tag="ones41", name="ones41")
            nc.vector.tensor_copy(ones41[:], o41f[:])
            zbias = cpool.tile([128, 1], f32, tag="zbias", name="zbias")
            nc.gpsimd.memset(zbias[:], 0.0)
            ident = cpool.tile([128, 128], bf16, tag="ident", name="ident")
            masks.make_identity(nc, ident[:])

            # ---- input DMAs, first-use order ----
            # x tensors on the Pool queue: xk split in 8 chunks (K proj
            # streams with the transfer), xq/xv as one batched DMA each.
            # weights/biases batched per tensor on the SP queue.
            x_c = {}
            for nm, d in (("xk", xk_d), ("xv", xv_d), ("xq", xq_d)):
                for kk in range(8):
                    t = cpool.tile([128, S], bf16, tag=f"{nm}c{kk}",
                                   name=f"{nm}c{kk}")
                    if do_xdma:
                        nc.gpsimd.dma_start(
                            t[:], d.ap()[128 * kk:128 * (kk + 1), :])
                    else:
                        nc.gpsimd.memset(t[:], 0.0)
                    x_c[(nm, kk)] = t

            def xap(nm, kk):
                return x_c[(nm, kk)][:]

            w_all = {}
            b_sb = {}
            w_all["wk"] = cpool.tile([128, 8, FG], bf16, tag="wka", name="wka")
            nc.sync.dma_start(w_all["wk"][:],
                              wk_d.ap().rearrange("(k p) c -> p k c", p=128))
            b_sb["bk"] = cpool.tile([128, 2, 1], f32, tag="bka", name="bka")
            nc.sync.dma_start(b_sb["bk"][:],
                              bk_d.ap().rearrange("(k p) c -> p k c", p=128))
            w_all["wv"] = cpool.tile([128, 8, FG], bf16, tag="wva", name="wva")
            nc.sync.dma_start(w_all["wv"][:],
                              wv_d.ap().rearrange("(k p) c -> p k c", p=128))
            b_sb["bv"] = cpool.tile([128, 2, 1], f32, tag="bva", name="bva")
            nc.sync.dma_start(b_sb["bv"][:],
                              bv_d.ap().rearrange("(k p) c -> p k c", p=128))
            w_all["wq"] = cpool.tile([128, 8, FG], bf16, tag="wqa", name="wqa")
            nc.sync.dma_start(w_all["wq"][:],
                              wq_d.ap().rearrange("(k p) c -> p k c", p=128))
            b_sb["bq"] = cpool.tile([128, 2, 1], f32, tag="bqa", name="bqa")
            nc.sync.dma_start(b_sb["bq"][:],
                              bq_d.ap().rearrange("(k p) c -> p k c", p=128))
            wo_all = cpool.tile([128, 2, D], bf16, tag="woa", name="woa")
            nc.sync.dma_start(wo_all[:],
                              wo_d.ap().rearrange("(k p) c -> p k c", p=128))

            def wap(nm, kk, pch):
                return w_all[nm][:, kk, 128 * pch:128 * (pch + 1)]

            def bap(nm, pch):
                return b_sb[nm][:, pch, :]

            # persistent activations
            qT = [cpool.tile([128, S], bf16, tag=f"qT{p}", name=f"qT{p}")
                  for p in range(2)]
            kT = [cpool.tile([128, S], bf16, tag=f"kT{p}", name=f"kT{p}")
                  for p in range(2)]
            vT = [cpool.tile([128, S], bf16, tag=f"vT{p}", name=f"vT{p}")
                  for p in range(2)]
            vI = [cpool.tile([128, 4, 65], bf16, tag=f"vI{sc}", name=f"vI{sc}")
                  for sc in range(16)]
            for sc in range(16):
                nc.vector.tensor_copy(vI[sc][:, :, 64:65], ones41[:])
            ctxN = [cpool.tile([128, S], bf16, tag=f"ctxN{p}", name=f"ctxN{p}")
                    for p in range(2)]

            # ---------------- phase A: K,V proj (kk-outer), transposes -----
            if do_proj:
                with tc.tile_pool(name="pa", bufs=1, space="PSUM") as papool:
                    for wnm, bnm, xnm, outT in (("wk", "bk", "xk", kT),
                                                ("wv", "bv", "xv", vT)):
                        ps = [papool.tile([128, 512], f32, name=f"pa{i}",
                                          bufs=1) for i in range(8)]
                        for kk in range(8):
                            for pch in range(2):
                                for qc in range(4):
                                    nc.tensor.matmul(
                                        ps[pch * 4 + qc][:],
                                        wap(wnm, kk, pch),
                                        xap(xnm, kk)[:, 512 * qc:512 * (qc + 1)],
                                        start=(kk == 0), stop=(kk == 7))
                        for pch in range(2):
                            for qc in range(4):
                                nc.scalar.activation(
                                    outT[pch][:, 512 * qc:512 * (qc + 1)],
                                    ps[pch * 4 + qc][:], IDENT,
                                    bias=bap(bnm, pch), scale=1.0)
                    # V transposes: 8 psum slots in flight
                    for sc in range(16):
                        for pch in range(2):
                            t = papool.tile([128, 512], f32,
                                            name=f"pa{(2 * sc + pch) % 8}",
                                            bufs=1)
                            tp = t[:, 0:64].bitcast(bf16)
                            nc.tensor.transpose(
                                tp, vT[pch][:, 128 * sc:128 * (sc + 1)],
                                ident[:])
                            nc.vector.tensor_copy(
                                vI[sc][:, 2 * pch:2 * pch + 2, 0:64],
                                tp.rearrange("p (h d) -> p h d", h=2))

            with tc.tile_pool(name="mp", bufs=1, space="PSUM") as mpool, \
                 tc.tile_pool(name="scp", bufs=1, space="PSUM") as scp, \
                 tc.tile_pool(name="cxp", bufs=1, space="PSUM") as cxp, \
                 tc.tile_pool(name="exa", bufs=8) as expa, \
                 tc.tile_pool(name="exd", bufs=8) as expd, \
                 tc.tile_pool(name="rcp", bufs=2) as rcpool, \
                 tc.tile_pool(name="csp", bufs=2) as cspool, \
                 tc.tile_pool(name="obp", bufs=2) as obpool:

                def proj_group(xnm, wnm, bnm, pch, qc, outT, bias_engine):
                    """[128,512] output chunk: 8 accumulating MMs + bias."""
                    p = mpool.tile([128, 512], f32, name="mp", bufs=2)
                    for kk in range(8):
                        if rt_proj:
                            for h in range(2):
                                nc.tensor.matmul(
                                    p[:],
                                    wap(wnm, kk, pch)[64 * h:64 * h + 64, :],
                                    xap(xnm, kk)[64 * h:64 * h + 64,
                                                 512 * qc:512 * (qc + 1)],
                                    start=(kk == 0 and h == 0),
                                    stop=(kk == 7 and h == 1),
                                    tile_position=(64 * h, 0))
                            continue
                        nc.tensor.matmul(
                            p[:], wap(wnm, kk, pch),
                            xap(xnm, kk)[:, 512 * qc:512 * (qc + 1)],
                            start=(kk == 0), stop=(kk == 7))
                    dst = outT[pch][:, 512 * qc:512 * (qc + 1)]
                    if bias_engine == "act":
                        nc.scalar.activation(dst, p[:], IDENT,
                                             bias=bap(bnm, pch), scale=1.0)
                    else:
                        nc.vector.tensor_scalar_add(dst, p[:], bap(bnm, pch))

                def qproj_step(qj, g):
                    """both pch groups advance one kk per iter (banks
                    alternate so consecutive matmuls pipeline)."""
                    if g == 0:
                        qproj_step.p = [mpool.tile([128, 512], f32, name="mp",
                                                   bufs=2) for _ in range(2)]
                    kk = g
                    for pch in range(2):
                        nc.tensor.matmul(
                            qproj_step.p[pch][:], wap("wq", kk, pch),
                            xap("xq", kk)[:, 512 * qj:512 * (qj + 1)],
                            start=(kk == 0), stop=(kk == 7))
                    if g == 7:
                        for pch in range(2):
                            nc.vector.tensor_scalar_add(
                                qT[pch][:, 512 * qj:512 * (qj + 1)],
                                qproj_step.p[pch][:], bap("bq", pch))

                def outproj_pair(qj, mbase, tail=False):
                    """two 128-row chunks of the output projection of qj,
                    interleaved so psum banks alternate per matmul."""
                    pool = scp if tail else mpool
                    nm = "sup" if tail else "mp"
                    ops = [pool.tile([128, 512], f32, name=nm, bufs=2)
                           for _ in range(2)]
                    for kk2 in range(2):
                        for t in range(2):
                            nc.tensor.matmul(
                                ops[t][:],
                                wo_all[:, kk2, 128 * (mbase + t):
                                       128 * (mbase + t + 1)],
                                ctxN[kk2][:, 512 * qj:512 * (qj + 1)],
                                start=(kk2 == 0), stop=(kk2 == 1))
                    for t in range(2):
                        m = mbase + t
                        ob = obpool.tile([128, 512], f32, name="ob", bufs=4)
                        nc.scalar.copy(ob[:], ops[t][:])
                        dma_eng = nc.gpsimd if m % 2 == 0 else nc.sync
                        dma_eng.dma_start(
                            out_d.ap()[128 * m:128 * (m + 1),
                                       512 * qj:512 * (qj + 1)],
                            ob[:])

                def attnv_mm(ctx, pk, pch, hh, mv, start, stop):
                    if rt_attnv:
                        for h in range(2):
                            nc.tensor.matmul(
                                ctx[:],
                                vI[pk][64 * h:64 * h + 64, 2 * pch + hh, :],
                                mv[64 * h:64 * h + 64, :],
                                start=(start and h == 0),
                                stop=(stop and h == 1),
                                tile_position=(64 * h, 0))
                    else:
                        nc.tensor.matmul(ctx[:], vI[pk][:, 2 * pch + hh, :],
                                         mv, start=start, stop=stop)

                # Q(qj=0) projection: both pch groups kk-interleaved
                if do_proj:
                    q0p = [mpool.tile([128, 512], f32, name="mp", bufs=2)
                           for _ in range(2)]
                    for kk in range(8):
                        for pch in range(2):
                            nc.tensor.matmul(
                                q0p[pch][:], wap("wq", kk, pch),
                                xap("xq", kk)[:, 0:512],
                                start=(kk == 0), stop=(kk == 7))
                    for pch in range(2):
                        nc.scalar.activation(qT[pch][:, 0:512], q0p[pch][:],
                                             IDENT, bias=bap("bq", pch),
                                             scale=1.0)

                # ---------------- attention + fused extra work ------------
                if do_attn:
                  for qj in range(4):
                    for hp in range(2):
                        pch = hp
                        ctxs = [cxp.tile([65, 512], f32, name=f"ctx{hh}",
                                         bufs=1) for hh in range(2)]
                        pend = None
                        for g in range(8):
                            if do_proj and hp == 1 and qj < 3:
                                qproj_step(qj + 1, g)
                            if hp == 0 and qj > 0 and g % 2 == 1:
                                outproj_pair(qj - 1, g - 1)

                            cur = []
                            sups = [scp.tile([128, 2, 512], f32, name="sup",
                                             bufs=2) for _ in range(2)]
                            # j-outer, hh-inner: adjacent matmuls target
                            # disjoint PE row groups, so each LDWEIGHTS
                            # overlaps the other head's in-flight matmul
                            for j in range(2):
                                ki = 2 * g + j
                                for hh in range(2):
                                    off = 64 * hh
                                    nc.tensor.matmul(
                                        sups[hh][:, j, :],
                                        kT[pch][off:off + 64,
                                                128 * ki:128 * (ki + 1)],
                                        qT[pch][off:off + 64,
                                                512 * qj:512 * (qj + 1)],
                                        start=True, stop=True,
                                        tile_position=(off, 0))
                            for hh in range(2):
                                sup = sups[hh]
                                if use_dve(qj, hp, g, hh):
                                    exd = expd.tile([128, 2, 512], u16,
                                                    name="exd", bufs=4)
                                    with nc.allow_low_precision(
                                            reason="schraudolph exp bits"):
                                        nc.vector.tensor_scalar(
                                            exd[:], sup[:],
                                            SCHR_A, schr_b_val,
                                            op0=MULT, op1=ADD)
                                    cur.append(exd[:].bitcast(bf16))
                                elif do_exp:
                                    exa = expa.tile([128, 2, 512], bf16,
                                                    name="exa", bufs=4)
                                    nc.scalar.activation(exa[:], sup[:], EXP,
                                                         bias=zbias[:],
                                                         scale=0.125)
                                    cur.append(exa[:])
                                else:
                                    cur.append(None)
                            if pend is not None:
                                # j-outer, hh-inner: ctx banks alternate per
                                # matmul so consecutive attn@V matmuls pipeline
                                for j in range(2):
                                    pk = 2 * (g - 1) + j
                                    for hh in range(2):
                                        mv = (pend[hh][:, j, :] if do_exp else
                                              qT[pch][:, 512 * qj:512 * (qj + 1)])
                                        attnv_mm(ctxs[hh], pk, pch, hh, mv,
                                                 start=(pk == 0), stop=False)
                            pend = cur
                        # tail attn@V + normalization, fused per head so the
                        # reciprocal of head 0 runs while head 1's tail matmuls
                        # are still on the PE (no PE wait on DVE at block end);
                        # ctx values stage to SBUF on ACT in parallel.  The two
                        # heads' denominator broadcasts share one PE matmul via
                        # the ones2 block stationary.
                        rcb = rc01[(2 * qj + hp) % 2]
                        css = []
                        for j in range(2):
                            pk = 14 + j
                            for hh in range(2):
                                mv = (pend[hh][:, j, :] if do_exp else
                                      qT[pch][:, 512 * qj:512 * (qj + 1)])
                                attnv_mm(ctxs[hh], pk, pch, hh, mv,
                                         start=False, stop=(pk == 15))
                        for hh in range(2):
                            with nc.allow_low_precision(
                                    reason="f32r for PE broadcast"):
                                nc.vector.reciprocal(rcb[64 * hh:64 * hh + 1, :],
                                                     ctxs[hh][64:65, :])
                            cs = cspool.tile([64, 512], f32, name="cs", bufs=4)
                            nc.scalar.copy(cs[:], ctxs[hh][0:64, :])
                            css.append(cs)
                        bc = scp.tile([128, 512], f32, name="sup", bufs=2)
                        nc.tensor.matmul(bc[:], ones2[:, :], rcb[:],
                                         start=True, stop=True)
                        for hh in range(2):
                            off = 64 * hh
                            nc.vector.tensor_tensor(
                                ctxN[pch][off:off + 64,
                                          512 * qj:512 * (qj + 1)],
                                css[hh][:], bc[64 * hh:64 * hh + 64, :], MULT)
                  # tail: output projection of the last qj chunk
                  for mb in range(0, 8, 2):
                      outproj_pair(3, mb, tail=True)

    nc.compile()
    return nc


def _make_in_maps(q, k, v, wq, bq, wk, bk, wv, bv, wo):
    bf = ml_dtypes.bfloat16
    in_maps = []
    for c in range(8):
        b, g = divmod(c, 4)
        fs = slice(FG * g, FG * (g + 1))
        in_maps.append({
            "xq": np.ascontiguousarray(q[b].T.astype(bf)),
            "xk": np.ascontiguousarray(k[b].T.astype(bf)),
            "xv": np.ascontiguousarray(v[b].T.astype(bf)),
            "wq": np.ascontiguousarray(wq[fs, :].T.astype(bf)),
            "wk": np.ascontiguousarray(wk[fs, :].T.astype(bf)),
            "wv": np.ascontiguousarray(wv[fs, :].T.astype(bf)),
            "wo": np.ascontiguousarray(wo[:, fs].T.astype(bf)),
            "bq": np.ascontiguousarray(bq[fs].reshape(FG, 1).astype(np.float32)),
            "bk": np.ascontiguousarray(bk[fs].reshape(FG, 1).astype(np.float32)),
            "bv": np.ascontiguousarray(bv[fs].reshape(FG, 1).astype(np.float32)),
        })
    return in_maps


def kernel(q, k, v, wq, bq, wk, bk, wv, bv, wo, bo):
    from concourse.bass_utils import run_bass_kernel_spmd

    global _compiled
    if _compiled is None:
        _compiled = _build_program()
    nc = _compiled

    args = [np.asarray(a, dtype=np.float32)
            for a in (q, k, v, wq, bq, wk, bk, wv, bv, wo)]
    bo = np.asarray(bo, dtype=np.float32)
    in_maps = _make_in_maps(*args)
    res = run_bass_kernel_spmd(nc, in_maps, core_ids=list(range(8)))
    outs = [np.asarray(res.results[c]["out"]) for c in range(8)]
    full = []
    for b in range(B):
        acc = outs[4 * b] + outs[4 * b + 1] + outs[4 * b + 2] + outs[4 * b + 3]
        full.append(acc.T + bo[None, :])
    return np.stack(full).astype(np.float32)


# revision 33
# speedup vs baseline: 5.9978x; 1.4376x over previous
"""MultiHeadAttention (B=2, S=2048, D=1024, H=16, depth=64) on 8 trn2 cores.

Sharding: core c -> batch b=c//4, head-group g=c%4 (heads 4g..4g+3).
Single fused pipeline per core:
  - host pre-transposes inputs to feature-major bf16 xq/xk/xv [1024, 2048];
    weights bf16; biases fp32 column vectors [256, 1].
  - x DMAs in 8 row-chunks each, first-use order xk,xv,xq on the gpsimd
    queue (projections stream with the transfers); weights batched per
    tensor on the SP queue.
  - Phase A: K proj kk-outer across 8 PSUM banks, then V proj kk-outer,
    then PE-transposes of V into seq-major interleaved tiles vI[sc]
    [128, 4, 65] with an all-ones column 64 per head (so attn@V also
    produces the softmax denominator), then Q(qj=0).
  - HW-critical ordering rules (measured on trn2): consecutive PE matmuls
    must alternate PSUM banks (same-bank back-to-back serializes, ~360 vs
    ~180 ns per N=512 matmul), and alternating 64-row tile_position groups
    lets LDWEIGHTS overlap the in-flight matmul (scores pairs ~130 ns/MM).
    Hence: scores emitted j-outer/hh-inner (row groups + banks alternate),
    attn@V j-outer/hh-inner (ctx banks alternate), Q-proj advances both
    pch groups one kk per step (two banks alternate), out-proj processes
    m-pairs with kk2-outer (two banks alternate).
  - attention processes head pairs with a one-step software-pipeline skew
    (attn@V of step g-1 issues after scores/exp of step g).
    Q(qj+1) projection interleaves into (qj, hp=1) g-steps; output
    projection of qj interleaves into (qj+1, hp=0) g-steps (one pair per
    odd step, so its ctxN dependency never stalls the PE at block entry).
  - exp is SPLIT 50/50 between ACT (true Exp activation, scale 1/8) and
    DVE (Schraudolph bf16 trick: y_bits = u16(23.083*s + B) viewed as
    bf16, ~1.5% max rel err); the softmax denominator stays consistent
    because it sums the same quantized weights via the vI ones column.
  - normalization: per-head reciprocal of ctx row 64 (DVE, f32r, into rows
    0/64 of a pre-zeroed carrier) -> one merged rank-1 PE matmul broadcast
    via a block stationary -> multiply (DVE) into bf16 ctxN [256, 2048];
    ctx values stage PSUM->SBUF on ACT in parallel.
  - out projection -> psum -> ACT copy to SBUF -> DMA (gpsimd/sync queues
    alternating) -> out partial [1024, 2048] fp32; host sums the 4
    head-group partials per batch, transposes back, adds bo.
"""

import numpy as np
import ml_dtypes

B, S, D = 2, 2048, 1024
FG = 256  # features per core (4 heads x 64)

# Schraudolph bf16 exp: bits = u16(A8*scores + SCHR_B); scores scale 1/8 folded
SCHR_A = (1 << 7) / float(np.log(2.0)) * 0.125
SCHR_B = 127.0 * (1 << 7) - (1 << 7) * 0.043 + 0.5  # +0.5 if convert truncates

# which exp tiles go to DVE: (2*g + hh) % DVE_MOD < DVE_CNT
DVE_MOD, DVE_CNT = 2, 1

_compiled = None


def _build_program(repeat=1, do_proj=True, do_attn=True, do_exp=True,
                   do_xdma=True, schr_b=None, rt_attnv=False, rt_proj=False):
    import concourse.bass as bass  # noqa: F401
    import concourse.tile as tile
    from concourse import bacc, mybir, masks

    f32 = mybir.dt.float32
    f32r = mybir.dt.float32r
    bf16 = mybir.dt.bfloat16
    u16 = mybir.dt.uint16
    EXP = mybir.ActivationFunctionType.Exp
    IDENT = mybir.ActivationFunctionType.Identity
    MULT = mybir.AluOpType.mult
    ADD = mybir.AluOpType.add
    schr_b_val = SCHR_B if schr_b is None else schr_b

    nc = bacc.Bacc("TRN2", target_bir_lowering=False, debug=False)

    xq_d = nc.dram_tensor("xq", [D, S], bf16, kind="ExternalInput")
    xk_d = nc.dram_tensor("xk", [D, S], bf16, kind="ExternalInput")
    xv_d = nc.dram_tensor("xv", [D, S], bf16, kind="ExternalInput")
    wq_d = nc.dram_tensor("wq", [D, FG], bf16, kind="ExternalInput")
    wk_d = nc.dram_tensor("wk", [D, FG], bf16, kind="ExternalInput")
    wv_d = nc.dram_tensor("wv", [D, FG], bf16, kind="ExternalInput")
    wo_d = nc.dram_tensor("wo", [FG, D], bf16, kind="ExternalInput")
    bq_d = nc.dram_tensor("bq", [FG, 1], f32, kind="ExternalInput")
    bk_d = nc.dram_tensor("bk", [FG, 1], f32, kind="ExternalInput")
    bv_d = nc.dram_tensor("bv", [FG, 1], f32, kind="ExternalInput")
    out_d = nc.dram_tensor("out", [D, S], f32, kind="ExternalOutput")

    def use_dve(qj, hp, g, hh):
        if not do_exp:
            return False
        return (2 * g + hh) % DVE_MOD < DVE_CNT

    with tile.TileContext(nc) as tc:
      for _rep in range(repeat):
        with tc.tile_pool(name="const", bufs=1) as cpool:
            # block stationary for the merged per-block-pair denominator
            # broadcast: row 0 -> cols 0..63, row 64 -> cols 64..127
            # (memset can't target f32r: build in f32, tensor_copy across)
            ones2f = cpool.tile([65, 512], f32, tag="ones2f", name="ones2f")
            nc.gpsimd.memset(ones2f[:], 0.0)
            nc.gpsimd.memset(ones2f[0:1, 0:64], 1.0)
            nc.gpsimd.memset(ones2f[64:65, 64:128], 1.0)
            ones2 = cpool.tile([65, 128], f32r, tag="ones2", name="ones2")
            nc.vector.tensor_copy(ones2[:], ones2f[:, 0:128])
            # pre-zeroed reciprocal carriers (rows 1..63 stay zero forever)
            rc01 = []
            for i in range(2):
                t = cpool.tile([65, 512], f32r, tag=f"rc01{i}", name=f"rc01{i}")
                nc.vector.tensor_copy(t[:], ones2f[:, 0:512])
                rc01.append(t)
            o41f = cpool.tile([128, 4, 1], f32, tag="o41f", name="o41f")
            nc.gpsimd.memset(o41f[:], 1.0)
            ones41 = cpool.tile([128, 4, 1], bf16, tag="ones41", name="ones41")
            nc.vector.tensor_copy(ones41[:], o41f[:])
            zbias = cpool.tile([128, 1], f32, tag="zbias", name="zbias")
            nc.gpsimd.memset(zbias[:], 0.0)
            ident = cpool.tile([128, 128], bf16, tag="ident", name="ident")
            masks.make_identity(nc, ident[:])

            # ---- input DMAs, first-use order ----
            # x tensors on the Pool queue: xk split in 8 chunks (K proj
            # streams with the transfer), xq/xv as one batched DMA each.
            # weights/biases batched per tensor on the SP queue.
            x_c = {}
            for nm, d in (("xk", xk_d), ("xv", xv_d), ("xq", xq_d)):
                for kk in range(8):
                    t = cpool.tile([128, S], bf16, tag=f"{nm}c{kk}",
                                   name=f"{nm}c{kk}")
                    if do_xdma:
                        nc.gpsimd.dma_start(
                            t[:], d.ap()[128 * kk:128 * (kk + 1), :])
                    else:
                        nc.gpsimd.memset(t[:], 0.0)
                    x_c[(nm, kk)] = t

            def xap(nm, kk):
                return x_c[(nm, kk)][:]

            w_all = {}
            b_sb = {}
            w_all["wk"] = cpool.tile([128, 8, FG], bf16, tag="wka", name="wka")
            nc.sync.dma_start(w_all["wk"][:],
                              wk_d.ap().rearrange("(k p) c -> p k c", p=128))
            b_sb["bk"] = cpool.tile([128, 2, 1], f32, tag="bka", name="bka")
            nc.sync.dma_start(b_sb["bk"][:],
                              bk_d.ap().rearrange("(k p) c -> p k c", p=128))
            w_all["wv"] = cpool.tile([128, 8, FG], bf16, tag="wva", name="wva")
            nc.sync.dma_start(w_all["wv"][:],
                              wv_d.ap().rearrange("(k p) c -> p k c", p=128))
            b_sb["bv"] = cpool.tile([128, 2, 1], f32, tag="bva", name="bva")
            nc.sync.dma_start(b_sb["bv"][:],
                              bv_d.ap().rearrange("(k p) c -> p k c", p=128))
            w_all["wq"] = cpool.tile([128, 8, FG], bf16, tag="wqa", name="wqa")
            nc.sync.dma_start(w_all["wq"][:],
                              wq_d.ap().rearrange("(k p) c -> p k c", p=128))
            b_sb["bq"] = cpool.tile([128, 2, 1], f32, tag="bqa", name="bqa")
            nc.sync.dma_start(b_sb["bq"][:],
                              bq_d.ap().rearrange("(k p) c -> p k c", p=128))
            wo_all = cpool.tile([128, 2, D], bf16, tag="woa", name="woa")
            nc.sync.dma_start(wo_all[:],
                              wo_d.ap().rearrange("(k p) c -> p k c", p=128))

            def wap(nm, kk, pch):
                return w_all[nm][:, kk, 128 * pch:128 * (pch + 1)]

            def bap(nm, pch):
                return b_sb[nm][:, pch, :]

            # persistent activations
            qT = [cpool.tile([128, S], bf16, tag=f"qT{p}", name=f"qT{p}")
                  for p in range(2)]
            kT = [cpool.tile([128, S], bf16, tag=f"kT{p}", name=f"kT{p}")
                  for p in range(2)]
            vT = [cpool.tile([128, S], bf16, tag=f"vT{p}", name=f"vT{p}")
                  for p in range(2)]
            vI = [cpool.tile([128, 4, 65], bf16, tag=f"vI{sc}", name=f"vI{sc}")
                  for sc in range(16)]
            for sc in range(16):
                nc.vector.tensor_copy(vI[sc][:, :, 64:65], ones41[:])
            ctxN = [cpool.tile([128, S], bf16, tag=f"ctxN{p}", name=f"ctxN{p}")
                    for p in range(2)]

            # ---------------- phase A: K,V proj (kk-outer), transposes -----
            if do_proj:
                with tc.tile_pool(name="pa", bufs=1, space="PSUM") as papool:
                    for wnm, bnm, xnm, outT in (("wk", "bk", "xk", kT),
                                                ("wv", "bv", "xv", vT)):
                        ps = [papool.tile([128, 512], f32, name=f"pa{i}",
                                          bufs=1) for i in range(8)]
                        for kk in range(8):
                            for pch in range(2):
                                for qc in range(4):
                                    nc.tensor.matmul(
                                        ps[pch * 4 + qc][:],
                                        wap(wnm, kk, pch),
                                        xap(xnm, kk)[:, 512 * qc:512 * (qc + 1)],
                                        start=(kk == 0), stop=(kk == 7))
                        for pch in range(2):
                            for qc in range(4):
                                nc.scalar.activation(
                                    outT[pch][:, 512 * qc:512 * (qc + 1)],
                                    ps[pch * 4 + qc][:], IDENT,
                                    bias=bap(bnm, pch), scale=1.0)
                    # V transposes: 8 psum slots in flight
                    for sc in range(16):
                        for pch in range(2):
                            t = papool.tile([128, 512], f32,
                                            name=f"pa{(2 * sc + pch) % 8}",
                                            bufs=1)
                            tp = t[:, 0:64].bitcast(bf16)
                            nc.tensor.transpose(
                                tp, vT[pch][:, 128 * sc:128 * (sc + 1)],
                                ident[:])
                            nc.vector.tensor_copy(
                                vI[sc][:, 2 * pch:2 * pch + 2, 0:64],
                                tp.rearrange("p (h d) -> p h d", h=2))

            with tc.tile_pool(name="mp", bufs=1, space="PSUM") as mpool, \
                 tc.tile_pool(name="scp", bufs=1, space="PSUM") as scp, \
                 tc.tile_pool(name="cxp", bufs=1, space="PSUM") as cxp, \
                 tc.tile_pool(name="exa", bufs=8) as expa, \
                 tc.tile_pool(name="exd", bufs=8) as expd, \
                 tc.tile_pool(name="csp", bufs=2) as cspool, \
                 tc.tile_pool(name="obp", bufs=2) as obpool:

                def qproj_step(qj, g):
                    """both pch groups advance one kk per iter (banks
                    alternate so consecutive matmuls pipeline)."""
                    if g == 0:
                        qproj_step.p = [mpool.tile([128, 512], f32, name="mp",
                                                   bufs=2) for _ in range(2)]
                    kk = g
                    for pch in range(2):
                        nc.tensor.matmul(
                            qproj_step.p[pch][:], wap("wq", kk, pch),
                            xap("xq", kk)[:, 512 * qj:512 * (qj + 1)],
                            start=(kk == 0), stop=(kk == 7))
                    if g == 7:
                        for pch in range(2):
                            nc.vector.tensor_scalar_add(
                                qT[pch][:, 512 * qj:512 * (qj + 1)],
                                qproj_step.p[pch][:], bap("bq", pch))

                def outproj_pair(qj, mbase, tail=False):
                    """two 128-row chunks of the output projection of qj,
                    interleaved so psum banks alternate per matmul."""
                    pool = scp if tail else mpool
                    nm = "sup" if tail else "mp"
                    ops = [pool.tile([128, 512], f32, name=nm, bufs=2)
                           for _ in range(2)]
                    for kk2 in range(2):
                        for t in range(2):
                            nc.tensor.matmul(
                                ops[t][:],
                                wo_all[:, kk2, 128 * (mbase + t):
                                       128 * (mbase + t + 1)],
                                ctxN[kk2][:, 512 * qj:512 * (qj + 1)],
                                start=(kk2 == 0), stop=(kk2 == 1))
                    for t in range(2):
                        m = mbase + t
                        ob = obpool.tile([128, 512], f32, name="ob", bufs=4)
                        nc.scalar.copy(ob[:], ops[t][:])
                        dma_eng = nc.gpsimd if m % 2 == 0 else nc.sync
                        dma_eng.dma_start(
                            out_d.ap()[128 * m:128 * (m + 1),
                                       512 * qj:512 * (qj + 1)],
                            ob[:])

                def attnv_mm(ctx, pk, pch, hh, mv, start, stop):
                    if rt_attnv:
                        for h in range(2):
                            nc.tensor.matmul(
                                ctx[:],
                                vI[pk][64 * h:64 * h + 64, 2 * pch + hh, :],
                                mv[64 * h:64 * h + 64, :],
                                start=(start and h == 0),
                                stop=(stop and h == 1),
                                tile_position=(64 * h, 0))
                    else:
                        nc.tensor.matmul(ctx[:], vI[pk][:, 2 * pch + hh, :],
                                         mv, start=start, stop=stop)

                # Q(qj=0) projection: both pch groups kk-interleaved
                if do_proj:
                    q0p = [mpool.tile([128, 512], f32, name="mp", bufs=2)
                           for _ in range(2)]
                    for kk in range(8):
                        for pch in range(2):
                            nc.tensor.matmul(
                                q0p[pch][:], wap("wq", kk, pch),
                                xap("xq", kk)[:, 0:512],
                                start=(kk == 0), stop=(kk == 7))
                    for pch in range(2):
                        nc.scalar.activation(qT[pch][:, 0:512], q0p[pch][:],
                                             IDENT, bias=bap("bq", pch),
                                             scale=1.0)

                # ---------------- attention + fused extra work ------------
                if do_attn:
                  for qj in range(4):
                    for hp in range(2):
                        pch = hp
                        ctxs = [cxp.tile([65, 512], f32, name=f"ctx{hh}",
                                         bufs=1) for hh in range(2)]
                        pend = None
                        for g in range(8):
                            if do_proj and hp == 1 and qj < 3:
                                qproj_step(qj + 1, g)
                            if hp == 0 and qj > 0 and g % 2 == 1:
                                outproj_pair(qj - 1, g - 1)

                            cur = []
                            sups = [scp.tile([128, 2, 512], f32, name="sup",
                                             bufs=2) for _ in range(2)]
                            # j-outer, hh-inner: adjacent matmuls target
                            # disjoint PE row groups, so each LDWEIGHTS
                            # overlaps the other head's in-flight matmul
                            for j in range(2):
                                ki = 2 * g + j
                                for hh in range(2):
                                    off = 64 * hh
                                    nc.tensor.matmul(
                                        sups[hh][:, j, :],
                                        kT[pch][off:off + 64,
                                                128 * ki:128 * (ki + 1)],
                                        qT[pch][off:off + 64,
                                                512 * qj:512 * (qj + 1)],
                                        start=True, stop=True,
                                        tile_position=(off, 0))
                            for hh in range(2):
                                sup = sups[hh]
                                if use_dve(qj, hp, g, hh):
                                    exd = expd.tile([128, 2, 512], u16,
                                                    name="exd", bufs=4)
                                    with nc.allow_low_precision(
                                            reason="schraudolph exp bits"):
                                        nc.vector.tensor_scalar(
                                            exd[:], sup[:],
                                            SCHR_A, schr_b_val,
                                            op0=MULT, op1=ADD)
                                    cur.append(exd[:].bitcast(bf16))
                                elif do_exp:
                                    exa = expa.tile([128, 2, 512], bf16,
                                                    name="exa", bufs=4)
                                    nc.scalar.activation(exa[:], sup[:], EXP,
                                                         bias=zbias[:],
                                                         scale=0.125)
                                    cur.append(exa[:])
                                else:
                                    cur.append(None)
                            if pend is not None:
                                # j-outer, hh-inner: ctx banks alternate per
                                # matmul so consecutive attn@V matmuls pipeline
                                for j in range(2):
                                    pk = 2 * (g - 1) + j
                                    for hh in range(2):
                                        mv = (pend[hh][:, j, :] if do_exp else
                                              qT[pch][:, 512 * qj:512 * (qj + 1)])
                                        attnv_mm(ctxs[hh], pk, pch, hh, mv,
                                                 start=(pk == 0), stop=False)
                            pend = cur
                        # tail attn@V + normalization, fused per head so the
                        # reciprocal of head 0 runs while head 1's tail matmuls
                        # are still on the PE (no PE wait on DVE at block end);
                        # ctx values stage to SBUF on ACT in parallel.  The two
                        # heads' denominator broadcasts share one PE matmul via
                        # the ones2 block stationary.
                        rcb = rc01[(2 * qj + hp) % 2]
                        css = []
                        for j in range(2):
                            pk = 14 + j
                            for hh in range(2):
                                mv = (pend[hh][:, j, :] if do_exp else
                                      qT[pch][:, 512 * qj:512 * (qj + 1)])
                                attnv_mm(ctxs[hh], pk, pch, hh, mv,
                                         start=False, stop=(pk == 15))
                        for hh in range(2):
                            with nc.allow_low_precision(
                                    reason="f32r for PE broadcast"):
                                nc.vector.reciprocal(rcb[64 * hh:64 * hh + 1, :],
                                                     ctxs[hh][64:65, :])
                            cs = cspool.tile([64, 512], f32, name="cs", bufs=4)
                            nc.scalar.copy(cs[:], ctxs[hh][0:64, :])
                            css.append(cs)
                        bc = scp.tile([128, 512], f32, name="sup", bufs=2)
                        nc.tensor.matmul(bc[:], ones2[:, :], rcb[:],
                                         start=True, stop=True)
                        for hh in range(2):
                            off = 64 * hh
                            nc.vector.tensor_tensor(
                                ctxN[pch][off:off + 64,
                                          512 * qj:512 * (qj + 1)],
                                css[hh][:], bc[64 * hh:64 * hh + 64, :], MULT)
                  # tail: output projection of the last qj chunk
                  for mb in range(0, 8, 2):
                      outproj_pair(3, mb, tail=True)

    nc.compile()
    return nc


def _make_in_maps(q, k, v, wq, bq, wk, bk, wv, bv, wo):
    bf = ml_dtypes.bfloat16
    in_maps = []
    for c in range(8):
        b, g = divmod(c, 4)
        fs = slice(FG * g, FG * (g + 1))
        in_maps.append({
            "xq": np.ascontiguousarray(q[b].T.astype(bf)),
            "xk": np.ascontiguousarray(k[b].T.astype(bf)),
            "xv": np.ascontiguousarray(v[b].T.astype(bf)),
            "wq": np.ascontiguousarray(wq[fs, :].T.astype(bf)),
            "wk": np.ascontiguousarray(wk[fs, :].T.astype(bf)),
            "wv": np.ascontiguousarray(wv[fs, :].T.astype(bf)),
            "wo": np.ascontiguousarray(wo[:, fs].T.astype(bf)),
            "bq": np.ascontiguousarray(bq[fs].reshape(FG, 1).astype(np.float32)),
            "bk": np.ascontiguousarray(bk[fs].reshape(FG, 1).astype(np.float32)),
            "bv": np.ascontiguousarray(bv[fs].reshape(FG, 1).astype(np.float32)),
        })
    return in_maps


def kernel(q, k, v, wq, bq, wk, bk, wv, bv, wo, bo):
    from concourse.bass_utils import run_bass_kernel_spmd

    global _compiled
    if _compiled is None:
        _compiled = _build_program()
    nc = _compiled

    args = [np.asarray(a, dtype=np.float32)
            for a in (q, k, v, wq, bq, wk, bk, wv, bv, wo)]
    bo = np.asarray(bo, dtype=np.float32)
    in_maps = _make_in_maps(*args)
    res = run_bass_kernel_spmd(nc, in_maps, core_ids=list(range(8)))
    outs = [np.asarray(res.results[c]["out"]) for c in range(8)]
    full = []
    for b in range(B):
        acc = outs[4 * b] + outs[4 * b + 1] + outs[4 * b + 2] + outs[4 * b + 3]
        full.append(acc.T + bo[None, :])
    return np.stack(full).astype(np.float32)
